# revision 1
# baseline (speedup 1.0000x reference)
"""Trainium2 Bass kernel for nn_Attention_11793980194868.

Conv3d(depthwise,k3)+BN -> QKV linear -> 6-head attention -> out proj.
Sharding: data-parallel over batch B=8, one batch element per NeuronCore.

Design vs baseline (chunk-major pipelined version):
  - the whole kernel is one chunk-major pipeline over 512-wide tq chunks:
    conv-q region -> q-projection chunk -> QK/exp/AV/normalize for all 3
    head pairs -> transposed out-projection, with PSUM pools shared across
    stages so conv (PE-heavy) overlaps exp (ACT-heavy)
  - x loaded with 3 flat 1MB DMAs + ACT Identity pad-copies instead of 48
    small strided DMAs (32B inner runs are descriptor-bound on real HW)
  - full bf16 pipeline (x, weights, intermediates, output) with fp32 PSUM;
    ~2.6x fewer staged host<->device bytes
  - x staged unpadded/transposed [3,128,4096] bf16; zero-pad on chip by
    DMAing h-planes into a pre-zeroed [128,18,18,18] tile
  - depthwise conv split by output REGION across engines: PE regions use
    diagonal-weight matmuls (diags built in one broadcast tensor_tensor
    per (name,ci)); DVE/Pool regions use one 3-free-dim STT per tap into
    fp16 accumulators (no cross-engine merge adds)
  - attention head pairs: QK^T K=64 issued as adjacent matmuls on row
    groups 0:64/64:128 (concurrent on PE sub-arrays); even head AV uses
    the [v|ones] M=65 trick (denominator in PSUM row 64); odd head AV
    writes partitions 64:128 via col groups 2-3 with a concurrent
    ones-column matmul on col group 0 for its denominator
  - softmax 1/denom broadcast across partitions with K=1 ones matmuls on
    PE (no DRAM roundtrip); reciprocals split DVE/ACT; normalize muls
    split DVE/Pool
  - output projection computed transposed (out^T[c,t]) so the proj bias
    is a per-partition ACT bias; out DRAM tensor is [C,T] bf16 and the
    host transposes back
"""

import os
import numpy as np

try:
    import concourse  # noqa: F401
except ImportError:  # harness environment fallback
    import sys

    sys.path.insert(0, "/opt/trn_rl_repo")

B, T, C = 8, 4096, 384
H, DH = 6, 64
NCI = 3  # channel tiles of 128
P = 128
TKV = 512
EPS = 1e-5
SCALE = float(C) ** -0.5
PD = 18  # padded spatial extent
N_CORES = 8

_TAPS = [(kh, kw, kd) for kh in range(3) for kw in range(3) for kd in range(3)]

# conv q: 8 regions of 512 (2 h-planes) per ci, engine per region (per ci).
# GPSIMD/Pool tensor ops fail the V3 ISA check on real HW -- PE/DVE only.
QR = [
    ["pe", "pe", "dve", "pe", "dve", "dve", "pe", "dve"],
    ["pe", "pe", "pe", "dve", "pe", "dve", "dve", "dve"],
    ["pe", "pe", "dve", "pe", "dve", "pe", "dve", "dve"],
]
# conv k/v: engine per (name, ci) -- whole [128,512] output
KVR = {
    ("k", 0): "pe", ("k", 1): "pe", ("k", 2): "pe",
    ("v", 0): "pe", ("v", 1): "pe", ("v", 2): "pe",
}


def build_program():
    import concourse.bacc as bacc
    import concourse.tile as tile
    from concourse import mybir

    dt = mybir.dt
    f32, bf16, fp16 = dt.float32, dt.bfloat16, dt.float16
    AF = mybir.ActivationFunctionType
    MULT, ADD = mybir.AluOpType.mult, mybir.AluOpType.add

    nc = bacc.Bacc(None)

    # ---- DRAM I/O (per core = one batch element) ----
    xT_d = nc.dram_tensor("xT", [NCI, P, T], bf16, kind="ExternalInput")
    wc_d = {
        "q": nc.dram_tensor("wcq", [NCI, P, 27], f32, kind="ExternalInput"),
        "k": nc.dram_tensor("wck", [NCI, P, 27], f32, kind="ExternalInput"),
        "v": nc.dram_tensor("wcv", [NCI, P, 27], f32, kind="ExternalInput"),
    }
    wcb_d = {  # same conv weights, bf16 [tap, 1] layout for diag builds
        "q": nc.dram_tensor("wcbq", [NCI, P, 27], bf16, kind="ExternalInput"),
        "k": nc.dram_tensor("wcbk", [NCI, P, 27], bf16, kind="ExternalInput"),
        "v": nc.dram_tensor("wcbv", [NCI, P, 27], bf16, kind="ExternalInput"),
    }
    ident_d = nc.dram_tensor("ident", [P, P], bf16, kind="ExternalInput")
    wq_d = nc.dram_tensor("wqT", [NCI, P, C], bf16, kind="ExternalInput")
    wk_d = nc.dram_tensor("wkT", [NCI, P, C], bf16, kind="ExternalInput")
    wv_d = nc.dram_tensor("wvT", [NCI, P, C], bf16, kind="ExternalInput")
    pj_d = nc.dram_tensor("projT", [NCI, P, C], bf16, kind="ExternalInput")
    bq_d = nc.dram_tensor("betaq", [NCI, P, 1], f32, kind="ExternalInput")
    bk_d = nc.dram_tensor("betak", [NCI, P, 1], f32, kind="ExternalInput")
    bv_d = nc.dram_tensor("betav", [1, C], f32, kind="ExternalInput")
    pb_d = nc.dram_tensor("projb", [NCI, P, 1], f32, kind="ExternalInput")
    out_d = nc.dram_tensor("out", [C, T], bf16, kind="ExternalOutput")

    with nc.allow_low_precision("bf16 pipeline, tolerance 2e-2"), tile.TileContext(nc) as tc:
        with (
            tc.tile_pool(name="consts", bufs=1) as cpool,
            tc.tile_pool(name="dg", bufs=7) as dgp,
            tc.tile_pool(name="acc", bufs=8) as accp,
            tc.tile_pool(name="qbn", bufs=6) as qbnp,
            tc.tile_pool(name="kvbn", bufs=2 * NCI) as kvbnp,
            tc.tile_pool(name="zq", bufs=NCI) as zqp,
            tc.tile_pool(name="zk", bufs=NCI) as zkp,
            tc.tile_pool(name="zv", bufs=4) as zvp,
            tc.tile_pool(name="at", bufs=6) as atp,
            tc.tile_pool(name="oT", bufs=6) as otp,
            tc.tile_pool(name="stg", bufs=8) as stgp,
            tc.tile_pool(name="outt", bufs=4) as outp,
            tc.tile_pool(name="psA", bufs=2, space="PSUM") as psA,
            tc.tile_pool(name="psB", bufs=4, space="PSUM") as psB,
            tc.tile_pool(name="psC", bufs=2, space="PSUM") as psC,
        ):
            # ---- x tiles: three dedicated padded buffers; only the border
            # shell needs zeroing (interior is fully overwritten by the
            # pad-copies), done with 6 small DVE memsets per buffer ----
            xp_bufs = []
            for i in range(NCI):
                t = cpool.tile([P, PD, PD, PD], bf16, tag=f"xp{i}", name=f"xp{i}")
                nc.vector.memset(t[:, 0, :, :], 0.0)
                nc.vector.memset(t[:, 17, :, :], 0.0)
                nc.vector.memset(t[:, 1:17, 0, :], 0.0)
                nc.vector.memset(t[:, 1:17, 17, :], 0.0)
                nc.vector.memset(t[:, 1:17, 1:17, 0], 0.0)
                nc.vector.memset(t[:, 1:17, 1:17, 17], 0.0)
                xp_bufs.append(t)

            ident = cpool.tile([P, P], bf16, tag="ident")
            nc.sync.dma_start(ident[:], ident_d[:])
            ones = cpool.tile([P, P], bf16, tag="ones")
            nc.vector.memset(ones[:], 1.0)

            wc_sb, wcb_sb = {}, {}
            for cname in ("q", "k", "v"):
                for ci in range(NCI):
                    t = cpool.tile([P, 27], f32, tag=f"wc_{cname}_{ci}", name=f"wc_{cname}_{ci}")
                    nc.sync.dma_start(t[:], wc_d[cname][ci])
                    wc_sb[(cname, ci)] = t
                    t2 = cpool.tile([P, 27, 1], bf16, tag=f"wcb_{cname}_{ci}", name=f"wcb_{cname}_{ci}")
                    nc.sync.dma_start(t2[:], wcb_d[cname][ci].rearrange("p t -> p t ()"))
                    wcb_sb[(cname, ci)] = t2

            def load3(d, tag):
                ts = []
                for ci in range(NCI):
                    t = cpool.tile([P, C], bf16, tag=f"{tag}{ci}")
                    nc.sync.dma_start(t[:], d[ci])
                    ts.append(t)
                return ts

            # flat 1MB DMA per ci into a staging tile, then pad-copies on
            # ACT (idle during startup) into the zeroed padded buffer
            xstage = []
            for ci in range(NCI):
                t = cpool.tile([P, T], bf16, tag=f"xs{ci}", name=f"xstage{ci}")
                nc.sync.dma_start(t[:], xT_d[ci])
                xstage.append(t)
            for ci in range(NCI):
                xv = xstage[ci][:].rearrange("p (a b c) -> p a b c", a=16, b=16)
                for hh in range(16):
                    nc.scalar.activation(
                        xp_bufs[ci][:, hh + 1, 1:17, 1:17], xv[:, hh, :, :], AF.Identity
                    )

            wq_sb = load3(wq_d, "wq")
            wk_sb = load3(wk_d, "wk")
            wv_sb = load3(wv_d, "wv")
            pj_sb = load3(pj_d, "pj")
            bq_sb, bk_sb, pb_sb = [], [], []
            for ci in range(NCI):
                t = cpool.tile([P, 1], f32, tag=f"bq{ci}", name=f"bq_{ci}")
                nc.sync.dma_start(t[:], bq_d[ci])
                bq_sb.append(t)
                t = cpool.tile([P, 1], f32, tag=f"bk{ci}", name=f"bk_{ci}")
                nc.sync.dma_start(t[:], bk_d[ci])
                bk_sb.append(t)
                t = cpool.tile([P, 1], f32, tag=f"pb{ci}", name=f"pb_{ci}")
                nc.sync.dma_start(t[:], pb_d[ci])
                pb_sb.append(t)
            bv_bc = cpool.tile([P, C], f32, tag="bvbc")
            nc.sync.dma_start(bv_bc[:], bv_d[0, :].partition_broadcast(P))

            kbn = [kvbnp.tile([P, TKV], bf16, tag="kvbn", name=f"kbn{ci}") for ci in range(NCI)]
            vbn = [kvbnp.tile([P, TKV], bf16, tag="kvbn", name=f"vbn{ci}") for ci in range(NCI)]

            def build_diags(cname, ci):
                dgall = dgp.tile([P, 27, P], bf16, tag="dgall", name=f"dg_{cname}_{ci}")
                nc.vector.tensor_tensor(
                    dgall[:],
                    ident[:].rearrange("p (a c) -> p a c", a=1).broadcast_to([P, 27, P]),
                    wcb_sb[(cname, ci)][:].broadcast_to([P, 27, P]),
                    MULT,
                )
                return dgall

            def conv_region_elem(eng, cname, ci, xp, dst_ap_3d, h0, nh, stride):
                """All 27 taps on DVE: dst = sum_tap w*x_shift (per h-plane)."""
                acc = accp.tile(
                    [P, nh, 16 // stride, 16 // stride], fp16,
                    tag=f"acc{nh}_{stride}", name=f"acc_{cname}_{ci}_{h0}",
                )
                for tap in range(27):
                    kh, kw, kd = _TAPS[tap]
                    wsc = wc_sb[(cname, ci)][:, tap : tap + 1]
                    for hh in range(nh):
                        xin = xp[
                            :,
                            kh + stride * (h0 + hh),
                            kw : kw + 16 : stride,
                            kd : kd + 16 : stride,
                        ]
                        if tap == 0:
                            # tap 0 on ACT (per-partition scale): offloads the
                            # one non-accumulating op from the saturated DVE
                            nc.scalar.activation(
                                acc[:, hh], xin, AF.Identity, scale=wsc
                            )
                        elif tap < 26:
                            eng.scalar_tensor_tensor(
                                acc[:, hh], xin, wsc, acc[:, hh], MULT, ADD
                            )
                        else:
                            eng.scalar_tensor_tensor(
                                dst_ap_3d[:, hh], xin, wsc, acc[:, hh], MULT, ADD
                            )

            # ---- q diag sets (kept live through the whole chunk loop) ----
            dgs_q = [build_diags("q", ci) for ci in range(NCI)]

            # ================= startup: k/v conv + projections =================
            # k side first; the v side is deferred until after chunk 0's
            # QK/exp issue so the ACT exp stream starts earlier
            def emit_kv_conv(name, dst_list):
                for ci in range(NCI):
                    dst = dst_list[ci]
                    eng_name = KVR[(name, ci)]
                    xp = xp_bufs[ci]
                    if eng_name == "pe":
                        dgs = build_diags(name, ci)
                        ps = psB.tile([P, 8, 8, 8], f32, tag="psB", name=f"ps{name}_{ci}")
                        for tap in range(27):
                            kh, kw, kd = _TAPS[tap]
                            nc.tensor.matmul(
                                ps[:],
                                dgs[:, tap, :],
                                xp[:, kh : kh + 16 : 2, kw : kw + 16 : 2, kd : kd + 16 : 2],
                                start=(tap == 0),
                                stop=(tap == 26),
                            )
                        nc.scalar.activation(
                            dst[:], ps[:].rearrange("p a b c -> p (a b c)"), AF.Identity
                        )
                    else:
                        eng = nc.vector if eng_name == "dve" else nc.gpsimd
                        conv_region_elem(
                            eng, name, ci, xp,
                            dst[:].rearrange("p (a b c) -> p a b c", a=8, b=8),
                            0, 8, 2,
                        )

            # ---- k projection: z_k^T[c_out, tkv] ----
            zk = []

            def emit_zk():
                for m in range(NCI):
                    z = zkp.tile([P, TKV], bf16, tag="zk", name=f"zk{m}")
                    zk.append(z)
                    ps = psC.tile([P, TKV], f32, tag="psC", name=f"zkps_{m}")
                    for kci in range(NCI):
                        nc.tensor.matmul(
                            ps[:],
                            wk_sb[kci][:, m * P : (m + 1) * P],
                            kbn[kci][:],
                            start=(kci == 0),
                            stop=(kci == NCI - 1),
                        )
                    nc.scalar.activation(z[:], ps[:], AF.Identity, bias=bk_sb[m][:, 0:1])

            # ---- v projection: z_v[tkv, (h, dh)] + ones column ----
            zv = []

            def emit_zv():
                for mt in range(4):
                    zt = zvp.tile([P, H, DH + 1], bf16, tag="zv", name=f"zv{mt}")
                    zv.append(zt)
                    ps = psC.tile([P, TKV], f32, tag="psC", name=f"zvps_{mt}")
                    for kci in range(NCI):
                        nc.tensor.matmul(
                            ps[:, 0:C],
                            vbn[kci][:, mt * P : (mt + 1) * P],
                            wv_sb[kci][:],
                            start=(kci == 0),
                            stop=(kci == NCI - 1),
                        )
                    nc.vector.tensor_add(
                        zt[:, :, 0:DH],
                        ps[:, 0:C].rearrange("p (h d) -> p h d", h=H),
                        bv_bc[:].rearrange("p (h d) -> p h d", h=H),
                    )
                    nc.vector.memset(zt[:, :, DH : DH + 1], 1.0)

            emit_kv_conv("k", kbn)
            emit_zk()

            # ================= chunk-major pipeline over tq =================
            # per 512-wide tq chunk r: conv-q region r (all ci) -> zq chunk ->
            # QK+exp+AV+normalize for all 3 head pairs -> transposed out proj
            for r in range(8):
                h0 = 2 * r
                # --- conv q region r on each ci ---
                qb_r = []
                for ci in range(NCI):
                    xp = xp_bufs[ci]
                    qb = qbnp.tile([P, TKV], bf16, tag="qbn", name=f"qbn_{r}_{ci}")
                    qb_r.append(qb)
                    if QR[ci][r] == "pe":
                        ps = psA.tile([P, 2, 16, 16], f32, tag="psA", name=f"psq_{r}_{ci}")
                        for tap in range(27):
                            kh, kw, kd = _TAPS[tap]
                            nc.tensor.matmul(
                                ps[:],
                                dgs_q[ci][:, tap, :],
                                xp[:, kh + h0 : kh + h0 + 2, kw : kw + 16, kd : kd + 16],
                                start=(tap == 0),
                                stop=(tap == 26),
                            )
                        nc.scalar.activation(
                            qb[:], ps[:].rearrange("p a b c -> p (a b c)"), AF.Identity
                        )
                    else:
                        conv_region_elem(
                            nc.vector, "q", ci, xp,
                            qb[:].rearrange("p (a b c) -> p a b c", a=2, b=16),
                            h0, 2, 1,
                        )

                # --- q projection for chunk r ---
                zq_r = []
                for m in range(NCI):
                    z = zqp.tile([P, TKV], bf16, tag="zq", name=f"zq_{r}_{m}")
                    zq_r.append(z)
                    ps = psC.tile([P, TKV], f32, tag="psC", name=f"zqps_{r}_{m}")
                    for kci in range(NCI):
                        nc.tensor.matmul(
                            ps[:],
                            wq_sb[kci][:, m * P : (m + 1) * P],
                            qb_r[kci][:],
                            start=(kci == 0),
                            stop=(kci == NCI - 1),
                        )
                    nc.scalar.activation(z[:], ps[:], AF.Identity, bias=bq_sb[m][:, 0:1])

                # --- attention for chunk r: QK+exp for all pairs first ---
                ats = []
                for hp in range(NCI):
                    at_e = atp.tile([P, 4, TKV], bf16, tag="at", name=f"ate_{r}_{hp}")
                    at_o = atp.tile([P, 4, TKV], bf16, tag="at", name=f"ato_{r}_{hp}")
                    ats.append((at_e, at_o))
                    for tkt in range(4):
                        ps_e = psA.tile([P, TKV], f32, tag="psA", name=f"qke_{r}_{hp}_{tkt}")
                        ps_o = psA.tile([P, TKV], f32, tag="psA", name=f"qko_{r}_{hp}_{tkt}")
                        nc.tensor.matmul(
                            ps_e[:],
                            zk[hp][0:64, tkt * P : (tkt + 1) * P],
                            zq_r[hp][0:64, :],
                            start=True, stop=True,
                        )
                        nc.tensor.matmul(
                            ps_o[:],
                            zk[hp][64:128, tkt * P : (tkt + 1) * P],
                            zq_r[hp][64:128, :],
                            start=True, stop=True,
                        )
                        nc.scalar.activation(at_e[:, tkt, :], ps_e[:], AF.Exp, scale=SCALE)
                        nc.scalar.activation(at_o[:, tkt, :], ps_o[:], AF.Exp, scale=SCALE)

                if r == 0:
                    # v side deferred to here: overlaps chunk 0's exps
                    emit_kv_conv("v", vbn)
                    emit_zv()

                # --- AV + normalize for all pairs ---
                oT_r = []
                for hp in range(NCI):
                    e, o = 2 * hp, 2 * hp + 1
                    at_e, at_o = ats[hp]
                    po_e = psB.tile([P, TKV], f32, tag="psB", name=f"poe_{r}_{hp}")
                    po_o = psB.tile([P, TKV], f32, tag="psB", name=f"poo_{r}_{hp}")
                    for tkt in range(4):
                        st, sp = (tkt == 0), (tkt == 3)
                        # even head: [v | ones] -> o_un rows 0:64, denom row 64
                        nc.tensor.matmul(
                            po_e[0:65, :], zv[tkt][:, e, :],
                            at_e[:, tkt, :], start=st, stop=sp,
                        )
                        # odd head: o_un rows 64:128 (col groups 2-3)
                        nc.tensor.matmul(
                            po_o[64:128, :], zv[tkt][:, o, 0:DH],
                            at_o[:, tkt, :], start=st, stop=sp,
                        )
                        # odd denom: ones column -> row 0 (col group 0)
                        nc.tensor.matmul(
                            po_o[0:1, :], ones[:, 0:1],
                            at_o[:, tkt, :], start=st, stop=sp,
                        )
                    stg = stgp.tile([P, TKV], bf16, tag="stg", name=f"stg_{r}_{hp}")
                    nc.vector.reciprocal(stg[64:65, :], po_e[64:65, :])
                    nc.vector.reciprocal(stg[0:1, :], po_o[0:1, :])
                    invb = psC.tile([P, TKV], f32, tag="psC", name=f"invb_{r}_{hp}")
                    nc.tensor.matmul(
                        invb[0:64, :], ones[64:65, 0:64], stg[64:65, :],
                        start=True, stop=True,
                    )
                    nc.tensor.matmul(
                        invb[64:128, :], ones[0:1, 0:64], stg[0:1, :],
                        start=True, stop=True,
                    )
                    # TensorTensor allows at most one PSUM operand: evacuate
                    invs = stgp.tile([P, TKV], bf16, tag="invs", name=f"invs_{r}_{hp}")
                    nc.scalar.activation(invs[:], invb[:], AF.Identity)
                    # evacuate AV PSUM to SBUF bf16 on ACT: frees the PSUM
                    # banks earlier and lets ONE all-SBUF bf16 mul (2x DVE
                    # rate) replace the two half-width PSUM muls
                    poev = stgp.tile([P, TKV], bf16, tag="poev", name=f"poev_{r}_{hp}")
                    nc.scalar.activation(poev[0:64, :], po_e[0:64, :], AF.Identity)
                    nc.scalar.activation(poev[64:128, :], po_o[64:128, :], AF.Identity)
                    o_t = otp.tile([P, TKV], bf16, tag="oT", name=f"oT_{r}_{hp}")
                    oT_r.append(o_t)
                    nc.vector.tensor_mul(o_t[:, :], poev[:, :], invs[:, :])

                # --- transposed output projection for chunk r ---
                for m in range(NCI):
                    ps = psC.tile([P, TKV], f32, tag="psC", name=f"pspj_{r}_{m}")
                    for kci in range(NCI):
                        nc.tensor.matmul(
                            ps[:],
                            pj_sb[kci][:, m * P : (m + 1) * P],
                            oT_r[kci][:],
                            start=(kci == 0),
                            stop=(kci == NCI - 1),
                        )
                    ot = outp.tile([P, TKV], bf16, tag="outt", name=f"ot_{r}_{m}")
                    nc.scalar.activation(ot[:], ps[:], AF.Identity, bias=pb_sb[m][:, 0:1])
                    nc.sync.dma_start(out_d[m * P : (m + 1) * P, r * TKV : (r + 1) * TKV], ot[:])

    nc.compile()
    return nc


def host_prep(inputs):
    """Fold BN, transpose x, cast to bf16, build per-core input maps."""
    import ml_dtypes

    f32 = np.float32
    bf16 = ml_dtypes.bfloat16
    x = np.asarray(inputs["x"], dtype=f32)

    def fold(p):
        g = np.asarray(inputs[f"bn_{p}_g"], f32)
        b = np.asarray(inputs[f"bn_{p}_b"], f32)
        m = np.asarray(inputs[f"bn_{p}_m"], f32)
        v = np.asarray(inputs[f"bn_{p}_v"], f32)
        a = g / np.sqrt(v + EPS)
        return a, b - m * a

    aq, bq = fold("q")
    ak, bk = fold("k")
    av_, bv = fold("v")

    wq = np.asarray(inputs["wq"], f32)
    wk = np.asarray(inputs["wk"], f32)
    wv = np.asarray(inputs["wv"], f32)
    pw = np.asarray(inputs["proj_w"], f32)
    pb = np.asarray(inputs["proj_b"], f32)

    wcq = np.ascontiguousarray(np.asarray(inputs["conv_q_w"], f32).reshape(NCI, P, 27))
    wck = np.ascontiguousarray(np.asarray(inputs["conv_k_w"], f32).reshape(NCI, P, 27))
    wcv = np.ascontiguousarray(np.asarray(inputs["conv_v_w"], f32).reshape(NCI, P, 27))

    common = {
        "wcq": wcq, "wck": wck, "wcv": wcv,
        "wcbq": wcq.astype(bf16), "wcbk": wck.astype(bf16), "wcbv": wcv.astype(bf16),
        "ident": np.eye(P, dtype=f32).astype(bf16),
        "wqT": np.ascontiguousarray((wq * aq[None, :]).T.reshape(NCI, P, C)).astype(bf16),
        "wkT": np.ascontiguousarray((wk * ak[None, :]).T.reshape(NCI, P, C)).astype(bf16),
        "wvT": np.ascontiguousarray((wv * av_[None, :]).T.reshape(NCI, P, C)).astype(bf16),
        "projT": np.ascontiguousarray(pw.T.reshape(NCI, P, C)).astype(bf16),
        "betaq": (wq @ bq).astype(f32).reshape(NCI, P, 1),
        "betak": (wk @ bk).astype(f32).reshape(NCI, P, 1),
        "betav": (wv @ bv).astype(f32).reshape(1, C),
        "projb": pb.astype(f32).reshape(NCI, P, 1),
    }

    # x: [B, T, C] -> per-batch channels-on-partitions [NCI, P, T] bf16
    xt = np.ascontiguousarray(x.transpose(0, 2, 1)).astype(bf16).reshape(B, NCI, P, T)

    in_maps = []
    for b in range(B):
        m = dict(common)
        m["xT"] = xt[b]
        in_maps.append(m)
    return in_maps


_CACHE = {}


def kernel(**inputs) -> np.ndarray:
    from concourse.bass_utils import run_bass_kernel_spmd

    if "nc" not in _CACHE:
        _CACHE["nc"] = build_program()
    nc = _CACHE["nc"]

    in_maps = host_prep(inputs)
    res = run_bass_kernel_spmd(
        nc,
        in_maps,
        core_ids=list(range(N_CORES)),
        trace=bool(int(os.environ.get("KERNEL_TRACE", "0"))),
    )
    # out is [C, T] bf16 per core; transpose back to [T, C] f32
    out = np.stack(
        [np.asarray(res.results[b]["out"]).astype(np.float32).T for b in range(B)], axis=0
    )
    _CACHE["last_result"] = res
    return out



# revision 4
# speedup vs baseline: 1.2838x; 1.2838x over previous
"""Trainium2 Bass kernel for nn_Attention_11793980194868 (v2 redesign).

Conv3d(depthwise,k3)+BN -> QKV linear -> 6-head attention -> out proj.
Sharding: data-parallel over batch B=8, one batch element per NeuronCore.

v2 design (vs chunk-major v1 at 326us):
  - AV issued stationary-flipped: lhsT = attn block [tkv,128tq], moving =
    [v|ones] [tkv,65] -> N=65 matmuls (cost-model charges output free size
    only), denominator rides as PSUM column 64; po layout [tq, 6head, 66]
    packs all heads in one PSUM bank
  - softmax normalize: reciprocal on [P,6] + one TensorTensor with
    broadcast inverse -> o_n [tq, 384]; no cross-partition broadcasts,
    no PSUM reciprocals of f512, no ACT evacuations
  - o_n transposed back to [c, t] via ONE dma_start_transpose per
    tq-block (XBAR, out[i,b,j]=in[j,128b+i]) into oT [P, 3, 512]
  - out-projection pipelined one chunk behind the transposes
  - conv: per-region tap split: taps [0,k) as PE diag matmuls into PSUM,
    taps [k,27) as DVE STT chain seeded from the PSUM partial (in1=psum),
    last tap writes qbn directly -> no separate evacuation
  - diag matrices DMA-loaded from DRAM (host-built), not built on DVE
  - exp applied per [P,512] PSUM bank -> at [P,4,2,512] bf16 per head pair
"""

import os
import numpy as np

try:
    import concourse  # noqa: F401
except ImportError:  # harness environment fallback
    import sys

    sys.path.insert(0, "/opt/trn_rl_repo")

B, T, C = 8, 4096, 384
H, DH = 6, 64
NCI = 3
P = 128
TKV = 512
EPS = 1e-5
SCALE = float(C) ** -0.5
PD = 18
N_CORES = 8

_TAPS = [(kh, kw, kd) for kh in range(3) for kw in range(3) for kd in range(3)]

# conv tap splits: taps [0, K_PE) on PE (diag matmuls, PSUM), rest on DVE
KQ_PE = 18   # per (ci, chunk-region) for conv q
KKV_PE = 27  # per (ci,) for conv k / conv v


def build_program():
    import concourse.bacc as bacc
    import concourse.tile as tile
    from concourse import mybir

    dt = mybir.dt
    f32, bf16, fp16 = dt.float32, dt.bfloat16, dt.float16
    AF = mybir.ActivationFunctionType
    MULT, ADD = mybir.AluOpType.mult, mybir.AluOpType.add

    nc = bacc.Bacc(None)

    # ---- DRAM I/O (per core = one batch element) ----
    xT_d = nc.dram_tensor("xT", [NCI, P, T], bf16, kind="ExternalInput")
    qdg_d = nc.dram_tensor("qdg", [P, NCI, 27, P], bf16, kind="ExternalInput")
    kdg_d = nc.dram_tensor("kdg", [P, NCI, 27, P], bf16, kind="ExternalInput")
    vdg_d = nc.dram_tensor("vdg", [P, NCI, 27, P], bf16, kind="ExternalInput")
    wc_d = nc.dram_tensor("wc", [P, 3, NCI, 27], f32, kind="ExternalInput")
    wall_d = nc.dram_tensor("wall", [P, 4, NCI, C], bf16, kind="ExternalInput")
    bpack_d = nc.dram_tensor("bpack", [P, NCI, 4], f32, kind="ExternalInput")
    bv_d = nc.dram_tensor("betav", [1, C], f32, kind="ExternalInput")
    out_d = nc.dram_tensor("out", [NCI, P, T], bf16, kind="ExternalOutput")

    from contextlib import ExitStack

    with ExitStack() as _es:
        _es.enter_context(nc.allow_low_precision("bf16 pipeline, tolerance 2e-2"))
        tc = _es.enter_context(tile.TileContext(nc))
        pool = lambda name, bufs, **kw: _es.enter_context(
            tc.tile_pool(name=name, bufs=bufs, **kw)
        )
        cpool = pool("consts", 1)
        kvdgp = pool("kvdg", 1)
        xsatp = pool("xsat", 6)   # xstage THEN at tiles
        qbnp = pool("qbn", 9)
        kvbnp = pool("kvbn", 6)
        accp = pool("acc", 4)
        zqp = pool("zq", 9)
        zkp = pool("zk", 3)
        zvp = pool("zv", 4)
        onp = pool("on", 3)
        invp = pool("inv", 3)
        otp = pool("oT", 3)
        outp = pool("outt", 2)
        psQK = pool("psQK", 3, space="PSUM")
        psCV = pool("psCV", 1, space="PSUM")
        psAV = pool("psAV", 2, space="PSUM")
        psPJ = pool("psPJ", 2, space="PSUM")
        if True:
            # ---- startup order: x + k-diags first (gate conv-k), rest after ----
            xstage = []
            for ci in range(NCI):
                t = xsatp.tile([P, 4096], bf16, tag="xsat", name=f"xstage{ci}")
                nc.sync.dma_start(t[:], xT_d[ci])
                xstage.append(t)
            kvdg = kvdgp.tile([P, NCI, 27, P], bf16, tag="kvdg", name="kdg")
            nc.sync.dma_start(kvdg[:], kdg_d[:])
            wc = cpool.tile([P, 3, NCI, 27], f32, tag="wc")
            nc.sync.dma_start(wc[:], wc_d[:])

            xp_bufs = []
            for ci in range(NCI):
                t = cpool.tile([P, PD, PD, PD], bf16, tag=f"xp{ci}", name=f"xp{ci}")
                nc.vector.memset(t[:, 0, :, :], 0.0)
                nc.vector.memset(t[:, 17, :, :], 0.0)
                nc.vector.memset(t[:, 1:17, 0, :], 0.0)
                nc.vector.memset(t[:, 1:17, 17, :], 0.0)
                nc.vector.memset(t[:, 1:17, 1:17, 0], 0.0)
                nc.vector.memset(t[:, 1:17, 1:17, 17], 0.0)
                xp_bufs.append(t)
            for ci in range(NCI):
                xv = xstage[ci][:].rearrange("p (a b c) -> p a b c", a=16, b=16)
                for hh in range(16):
                    dst = xp_bufs[ci][:, hh + 1, 1:17, 1:17]
                    if ci == 1:
                        nc.vector.tensor_copy(dst, xv[:, hh])
                    else:
                        nc.scalar.activation(dst, xv[:, hh], AF.Identity)

            qdg = cpool.tile([P, NCI, 27, P], bf16, tag="qdg")
            nc.sync.dma_start(qdg[:], qdg_d[:])
            wall = cpool.tile([P, 4, NCI, C], bf16, tag="wall")
            nc.sync.dma_start(wall[:], wall_d[:])
            bpack = cpool.tile([P, NCI, 4], f32, tag="bpack")
            nc.sync.dma_start(bpack[:], bpack_d[:])
            bv_bc = cpool.tile([P, C], f32, tag="bvbc")
            nc.sync.dma_start(bv_bc[:], bv_d[0, :].partition_broadcast(P))

            # weight slices
            def w_lhsT(which, kci, m):
                return wall[:, which, kci, m * P : (m + 1) * P]

            def bias_ap(which, m):
                return bpack[:, m, which : which + 1]

            # ---- conv helper: taps [0,k) PE -> PSUM, [k,27) DVE chain ----
            # STT ops must be <=3D: the DVE chain runs per h-plane.
            def conv_block(dg, which, ci, dst, h0, nh, stride, k_pe, tag):
                """dst: bf16 tile; output block [P, nh, 16/s, 16/s]."""
                xp = xp_bufs[ci]
                s = stride
                e = 16 // s
                w = nh * e * e
                d4 = dst[:].rearrange("p (a b c) -> p a b c", a=nh, b=e)
                ps = psCV.tile([P, TKV], f32, tag="psCV", name=f"cv_{tag}")
                ps4 = ps[:, 0:w].rearrange("p (a b c) -> p a b c", a=nh, b=e)
                for tap in range(k_pe):
                    kh, kw, kd = _TAPS[tap]
                    nc.tensor.matmul(
                        ps4,
                        dg[:, ci, tap, :],
                        xp[:, kh + s * h0 : kh + s * (h0 + nh) : s,
                           kw : kw + 16 : s, kd : kd + 16 : s],
                        start=(tap == 0),
                        stop=(tap == k_pe - 1),
                    )
                if k_pe == 27:
                    nc.scalar.activation(dst[:], ps[:, 0:w], AF.Identity)
                    return
                acc = accp.tile([P, TKV], fp16, tag="acc", name=f"acc_{tag}")
                a4 = acc[:, 0:w].rearrange("p (a b c) -> p a b c", a=nh, b=e)
                for hh in range(nh):
                    for tap in range(k_pe, 27):
                        kh, kw, kd = _TAPS[tap]
                        xin = xp[:, kh + s * (h0 + hh),
                                 kw : kw + 16 : s, kd : kd + 16 : s]
                        wsc = wc[:, which, ci, tap : tap + 1]
                        src = ps4[:, hh] if tap == k_pe else a4[:, hh]
                        dstap = d4[:, hh] if tap == 26 else a4[:, hh]
                        nc.vector.scalar_tensor_tensor(dstap, xin, wsc, src, MULT, ADD)

            # ---- k conv + k projection (startup) ----
            kbn = []
            for ci in range(NCI):
                t = kvbnp.tile([P, TKV], bf16, tag="kvbn", name=f"kbn{ci}")
                kbn.append(t)
                conv_block(kvdg, 1, ci, t, 0, 8, 2, KKV_PE, f"k{ci}")
            zk = []
            for m in range(NCI):
                z = zkp.tile([P, TKV], bf16, tag="zk", name=f"zk{m}")
                zk.append(z)
                ps = psPJ.tile([P, TKV], f32, tag="psPJ", name=f"zkps{m}")
                for kci in range(NCI):
                    nc.tensor.matmul(
                        ps[:], w_lhsT(1, kci, m), kbn[kci][:],
                        start=(kci == 0), stop=(kci == NCI - 1),
                    )
                nc.scalar.activation(z[:], ps[:], AF.Identity, bias=bias_ap(1, m))

            zv = []

            def emit_v():
                # v diags overwrite the k diag tile (WAR serialized by tile fw)
                vdg = kvdgp.tile([P, NCI, 27, P], bf16, tag="kvdg", name="vdg")
                nc.sync.dma_start(vdg[:], vdg_d[:])
                vbn = []
                for ci in range(NCI):
                    t = kvbnp.tile([P, TKV], bf16, tag="kvbn", name=f"vbn{ci}")
                    vbn.append(t)
                    conv_block(vdg, 2, ci, t, 0, 8, 2, KKV_PE, f"v{ci}")
                for mt in range(4):
                    zt = zvp.tile([P, H, DH + 1], bf16, tag="zv", name=f"zv{mt}")
                    zv.append(zt)
                    ps = psPJ.tile([P, TKV], f32, tag="psPJ", name=f"zvps{mt}")
                    for kci in range(NCI):
                        nc.tensor.matmul(
                            ps[:, 0:C],
                            vbn[kci][:, mt * P : (mt + 1) * P],
                            wall[:, 2, kci, :],
                            start=(kci == 0), stop=(kci == NCI - 1),
                        )
                    nc.vector.tensor_tensor(
                        zt[:, :, 0:DH],
                        ps[:, 0:C].rearrange("p (h d) -> p h d", h=H),
                        bv_bc[:].rearrange("p (h d) -> p h d", h=H),
                        ADD,
                    )
                    nc.vector.memset(zt[:, :, DH : DH + 1], 1.0)

            # ---- out-projection for a finished chunk (pipelined behind) ----
            def emit_outproj(r, t0, W, oT):
                ott = outp.tile([P, NCI, W], bf16, tag="outt", name=f"ot{r}")
                for m in range(NCI):
                    ps = psPJ.tile([P, TKV], f32, tag="psPJ", name=f"opj_{r}_{m}")
                    for kci in range(NCI):
                        nc.tensor.matmul(
                            ps[:, 0:W], w_lhsT(3, kci, m), oT[:, kci, :],
                            start=(kci == 0), stop=(kci == NCI - 1),
                        )
                    nc.scalar.activation(
                        ott[:, m, :], ps[:, 0:W], AF.Identity, bias=bias_ap(3, m)
                    )
                nc.sync.dma_start(
                    out_d[:, :, t0 : t0 + W].rearrange("a p t -> p a t"),
                    ott[:],
                )

            # ---- AV thunks for a chunk whose exps are already done ----
            def av_thunks(r, ats, nh):
                thunks = []
                pos = [None]

                def alloc(tqb):
                    pos[0] = psAV.tile([P, H, 66], f32, tag="psAV", name=f"po_{r}_{tqb}")

                for tqb in range(2 * nh):
                    for h in range(H):
                        hp, par = h // 2, h % 2
                        for tkt in range(4):
                            def mk(tqb=tqb, h=h, hp=hp, par=par, tkt=tkt):
                                if h == 0 and tkt == 0:
                                    alloc(tqb)
                                nc.tensor.matmul(
                                    pos[0][:, h, 0:65],
                                    ats[hp][:, tkt, par, tqb * P : (tqb + 1) * P],
                                    zv[tkt][:, h, :],
                                    start=(tkt == 0), stop=(tkt == 3),
                                )
                                if h == H - 1 and tkt == 3:
                                    return pos[0]
                                return None
                            thunks.append(mk)
                return thunks

            def finish_av(r, po_list, oT):
                for tqb, po in enumerate(po_list):
                    inv = invp.tile([P, H, 1], bf16, tag="inv", name=f"inv_{r}_{tqb}")
                    nc.vector.reciprocal(inv[:, :, 0], po[:, :, 64:65][:, :, 0])
                    on = onp.tile([P, H, DH], bf16, tag="on", name=f"on_{r}_{tqb}")
                    nc.vector.tensor_tensor(
                        on[:], po[:, :, 0:DH], inv[:].broadcast_to([P, H, DH]), MULT
                    )
                    nc.sync.dma_start_transpose(
                        oT[:, :, tqb * P : (tqb + 1) * P],
                        on[:].rearrange("p h d -> p (h d)"),
                    )

            # ================= chunk-major pipeline over tq =================
            # chunks: 7 full (2 h-planes / 512 tq) + 2 half (tail-latency trim)
            CHUNKS = [(2 * i, 2) for i in range(7)] + [(14, 1), (15, 1)]
            pend_av = None    # (r, ats, nh): AV runs during the next chunk
            pend_out = None   # (r, t0, W, oT) awaiting out-projection
            for r, (h0, nh) in enumerate(CHUNKS):
                W = 256 * nh
                t0 = 256 * h0
                if pend_out is not None:
                    emit_outproj(*pend_out)
                    pend_out = None

                # --- conv q region r (3 ci) ---
                qb_r = []
                for ci in range(NCI):
                    qb = qbnp.tile([P, W], bf16, tag="qbn", name=f"qbn_{r}_{ci}")
                    qb_r.append(qb)
                    conv_block(qdg, 0, ci, qb, h0, nh, 1, KQ_PE, f"q{r}{ci}")

                # --- q projection ---
                zq_r = []
                for m in range(NCI):
                    z = zqp.tile([P, W], bf16, tag="zq", name=f"zq_{r}_{m}")
                    zq_r.append(z)
                    ps = psPJ.tile([P, TKV], f32, tag="psPJ", name=f"zqps_{r}_{m}")
                    for kci in range(NCI):
                        nc.tensor.matmul(
                            ps[:, 0:W], w_lhsT(0, kci, m), qb_r[kci][:],
                            start=(kci == 0), stop=(kci == NCI - 1),
                        )
                    nc.scalar.activation(
                        z[:], ps[:, 0:W], AF.Identity, bias=bias_ap(0, m)
                    )

                # --- QK^T + exp, interleaved with AV of the previous chunk ---
                av = av_thunks(*pend_av) if pend_av is not None else []
                stride = -(-len(av) // 24) if av else 0
                po_list = []
                ai = 0
                ats = []
                for hp in range(NCI):
                    at = xsatp.tile([P, 4, 2, W], bf16, tag="xsat", name=f"at_{r}_{hp}")
                    ats.append(at)
                    for tkt in range(4):
                        for par in range(2):
                            rows = slice(64 * par, 64 * par + 64)
                            ps = psQK.tile(
                                [P, TKV], f32, tag="psQK", name=f"qk_{r}_{hp}_{tkt}_{par}"
                            )
                            nc.tensor.matmul(
                                ps[:, 0:W],
                                zk[hp][rows, tkt * P : (tkt + 1) * P],
                                zq_r[hp][rows, :],
                                start=True, stop=True,
                            )
                            nc.scalar.activation(
                                at[:, tkt, par, :], ps[:, 0:W], AF.Exp, scale=SCALE
                            )
                            for _ in range(stride):
                                if ai < len(av):
                                    po = av[ai]()
                                    if po is not None:
                                        po_list.append(po)
                                    ai += 1
                while ai < len(av):
                    po = av[ai]()
                    if po is not None:
                        po_list.append(po)
                    ai += 1

                if pend_av is not None:
                    rp, _, nhp = pend_av
                    oT = otp.tile([P, NCI, 256 * nhp], bf16, tag="oT", name=f"oT_{rp}")
                    finish_av(rp, po_list, oT)
                    pend_out = (rp, 256 * CHUNKS[rp][0], 256 * nhp, oT)

                if r == 0:
                    emit_v()
                pend_av = (r, ats, nh)

            # --- tail: AV(last), norm, final out-projections ---
            if pend_out is not None:
                emit_outproj(*pend_out)
                pend_out = None
            rl, _, nhl = pend_av
            av = av_thunks(*pend_av)
            po_list = []
            for th in av:
                po = th()
                if po is not None:
                    po_list.append(po)
            oT = otp.tile([P, NCI, 256 * nhl], bf16, tag="oT", name=f"oT_{rl}")
            finish_av(rl, po_list, oT)
            emit_outproj(rl, 256 * CHUNKS[rl][0], 256 * nhl, oT)

    nc.compile()
    return nc


def host_prep(inputs):
    """Fold BN, transpose x, build diag matrices, cast to bf16."""
    import ml_dtypes

    f32 = np.float32
    bf16 = ml_dtypes.bfloat16
    x = np.asarray(inputs["x"], dtype=f32)

    def fold(p):
        g = np.asarray(inputs[f"bn_{p}_g"], f32)
        b = np.asarray(inputs[f"bn_{p}_b"], f32)
        m = np.asarray(inputs[f"bn_{p}_m"], f32)
        v = np.asarray(inputs[f"bn_{p}_v"], f32)
        a = g / np.sqrt(v + EPS)
        return a, b - m * a

    aq, bq = fold("q")
    ak, bk = fold("k")
    av_, bv = fold("v")

    wq = np.asarray(inputs["wq"], f32)
    wk = np.asarray(inputs["wk"], f32)
    wv = np.asarray(inputs["wv"], f32)
    pw = np.asarray(inputs["proj_w"], f32)
    pb = np.asarray(inputs["proj_b"], f32)

    wcq = np.asarray(inputs["conv_q_w"], f32).reshape(C, 27)
    wck = np.asarray(inputs["conv_k_w"], f32).reshape(C, 27)
    wcv = np.asarray(inputs["conv_v_w"], f32).reshape(C, 27)

    def diags(w):
        # [P, NCI, 27, P]: dg[p, ci, tap, m] = w[ci*128+p, tap] if m==p
        dg = np.zeros((P, NCI, 27, P), f32)
        idx = np.arange(P)
        for ci in range(NCI):
            dg[idx, ci, :, idx] = w[ci * P : (ci + 1) * P, :]
        return dg.astype(bf16)

    # wall[p, which, kci, m] = W'[m, kci*128+p]  (lhsT layout [c_in, c_out])
    wall = np.zeros((P, 4, NCI, C), f32)
    for which, (w, a) in enumerate(
        [(wq, aq), (wk, ak), (wv, av_), (pw, None)]
    ):
        wT = (w * a[None, :]).T if a is not None else w.T  # [c_in, c_out]
        wall[:, which, :, :] = wT.reshape(NCI, P, C).transpose(1, 0, 2)

    # wc[p, which(q,k,v), ci, tap]
    wcp = np.zeros((P, 3, NCI, 27), f32)
    for which, w in enumerate([wcq, wck, wcv]):
        wcp[:, which, :, :] = w.reshape(NCI, P, 27).transpose(1, 0, 2)

    # bpack[p, m, which]: (bq, bk, pb, 0) for output block m
    bpk = np.zeros((P, NCI, 4), f32)
    bpk[:, :, 0] = (wq @ bq).reshape(NCI, P).T
    bpk[:, :, 1] = (wk @ bk).reshape(NCI, P).T
    bpk[:, :, 3] = pb.reshape(NCI, P).T

    common = {
        "qdg": diags(wcq), "kdg": diags(wck), "vdg": diags(wcv),
        "wc": wcp, "wall": wall.astype(bf16), "bpack": bpk,
        "betav": (wv @ bv).astype(f32).reshape(1, C),
    }

    xt = np.ascontiguousarray(x.transpose(0, 2, 1)).astype(bf16).reshape(B, NCI, P, T)
    in_maps = []
    for b in range(B):
        m = dict(common)
        m["xT"] = xt[b]
        in_maps.append(m)
    return in_maps


_CACHE = {}


def kernel(**inputs) -> np.ndarray:
    from concourse.bass_utils import run_bass_kernel_spmd

    if "nc" not in _CACHE:
        _CACHE["nc"] = build_program()
    nc = _CACHE["nc"]

    in_maps = host_prep(inputs)
    res = run_bass_kernel_spmd(
        nc,
        in_maps,
        core_ids=list(range(N_CORES)),
        trace=bool(int(os.environ.get("KERNEL_TRACE", "0"))),
    )
    out = np.stack(
        [
            np.asarray(res.results[b]["out"])
            .astype(np.float32)
            .reshape(C, T)
            .T
            for b in range(B)
        ],
        axis=0,
    )
    _CACHE["last_result"] = res
    return out


if __name__ == "__main__":
    import sys

    sys.path.insert(0, "/opt/trn_rl_repo")
    nc = build_program()
    from concourse.timeline_sim import TimelineSim

    est = TimelineSim(nc, trace=False).simulate()
    print(f"TimelineSim: {est:.0f} ns")
